# revision 1
# baseline (speedup 1.0000x reference)
"""Trainium2 Bass kernel for nn_MultiHeadAttention_46093589021200.

Causal MHA: B=4, S=2048, E=1024, H=16, D=64, with the reference's
"no-transpose-back" reshape (b,h,s,d)->(b,s,e) before the output projection.

Sharding: pure head-parallel, 2 heads per core, zero collectives.
Because of the reshape quirk, output rows s' in [h*128,(h+1)*128) depend only
on head h, so each core produces two independent 128-row output bands per
batch.

v2 design notes (vs the v1 baseline; 344250ns -> 241624ns):
  - q/k projected via PE into qkT [d2, s] head-major; v projected separately
    in NATURAL [s, d] layout (lhsT = x^T chunk), which is exactly the PV lhsT
    layout -> no DMA xbar transposes at all.
  - v bias folded into an effective o_proj bias on host (softmax rows sum to
    1, so + bv commutes through the attention average); o_proj bias added by
    DVE during the PSUM->SBUF copy (no K=1 bias matmuls).
  - attention in 512-wide q bands: scoresT [k,q] per 128-k chunk, the two
    heads in the two separate banks of one [128,1024] PSUM tile; one exp ACT
    instr per chunk covering both heads; triangular fp16 mask multiply on
    diagonal chunks; PV with v_aug stationary producing att [128, q] where
    v_aug = [v_h(64) | ones(64)] so rows 64-127 carry the rowsum replicated
    64x -> a single DVE reciprocal yields the normalization broadcast
    (no gpsimd partition_broadcast on the band-boundary critical path).
  - normalization DVE muls scatter normalized attn DIRECTLY into the o_proj
    "pair" layout: partition p = (w%2)*64 + d, column = (w//2)*128 + u for
    output row u, with q = u*16 + w. o_proj then runs K=128 matmuls (two
    w-blocks per MM) against untouched Wo row-chunks, halving o_proj columns.
  - PSUM: 4 score banks (2 tiles x 2) + 2 att + 2 accumulator = 8.
  - software-pipelined emission: bands(b) | proj(b+1) | ... with ALL o_proj
    emitted last as PE filler-of-last-resort; per-ec xt tiles (bufs=3) and
    half-tile first-batch DMA pieces (transfer-bound, not HWDGE-bound) keep
    next-batch projection chains ready to fill the ACT-bound attention gaps.
  - DMA ordering is the schedule: wqk | xt(0) | wv | xt(1) | wo+boeff |
    xt(2) | ... on the SP ring; tiny consts on the Pool/SWDGE ring.

HW rules learned by probing (CoreSim accepts all of these, HW does not):
  - matmuls from DIFFERENT PE row groups must not write the same PSUM bank
    (same-row-group region-sharing of a bank is fine).
  - column-positioned matmuls (tile_position=(0,32j), PSUM output at a
    partition offset) mis-execute. Matmul lhsT/rhs share their SBUF base
    partition. DVE ops MAY write partition-shifted outputs.
"""

import sys

if "/opt/trn_rl_repo" not in sys.path:
    sys.path.insert(0, "/opt/trn_rl_repo")

import numpy as np

B, S, E, H = 4, 2048, 1024, 16
D = E // H          # 64
NCORES = 8
HPC = H // NCORES   # heads per core = 2
SCALE = 1.0 / float(np.sqrt(D))
NB = S // 256       # 8 bands of 256 queries
NC = S // 128       # 16 key chunks

_CACHE = {}


def _build_program():
    import concourse.bass as bass  # noqa: F401
    import concourse.tile as tile
    from concourse import bacc, mybir

    f16 = mybir.dt.float16
    f32 = mybir.dt.float32
    Exp = mybir.ActivationFunctionType.Exp

    nc = bacc.Bacc("TRN2", target_bir_lowering=False, debug=False)

    xT = nc.dram_tensor("xT", [B, E, S], f16, kind="ExternalInput")
    wqk = nc.dram_tensor("wqk", [E, 256], f16, kind="ExternalInput")
    wv = nc.dram_tensor("wv", [E, 128], f16, kind="ExternalInput")
    bqk = nc.dram_tensor("bqk", [128, 2], f32, kind="ExternalInput")
    wo = nc.dram_tensor("wo", [E, E], f16, kind="ExternalInput")
    boeff = nc.dram_tensor("boeff", [128, 2 * E], f32, kind="ExternalInput")
    trimask2 = nc.dram_tensor("trimask2", [128, 256], f16, kind="ExternalInput")
    out = nc.dram_tensor("out", [B, HPC, 128, E], f32, kind="ExternalOutput")

    with tile.TileContext(nc) as tc:
        with (
            tc.tile_pool(name="const", bufs=1) as cp,
            tc.tile_pool(name="sb", bufs=2) as sb,
            tc.tile_pool(name="sb3", bufs=3) as sb3,
            tc.tile_pool(name="ps", bufs=2, space="PSUM") as ps,
        ):
            # ---- constants resident in SBUF for the whole kernel ----
            # ec0 slice first (364ns) so the first matmul unblocks early;
            # the remainder queues behind the first xt piece
            wqk_sb = cp.tile([128, 8 * 256], f16)     # [p, ec*256 + col]
            wqk_dram = wqk.ap().rearrange("(ec p) c -> p ec c", p=128)
            nc.sync.dma_start(wqk_sb[:, 0:256], wqk_dram[:, 0])
            # wv/bqk/trimask DMAs are deferred until after xt(0) (see below)
            wv_sb = cp.tile([128, 8 * 128], f16)      # [p, ec*128 + vcol]
            bqk_sb = cp.tile([128, 2], f32)
            trimask_sb = cp.tile([128, 256], f16)
            # o_proj constants allocated here, DMA'd after the prologue so
            # they don't delay xt(0) on the shared DMA engines
            wo_sb = cp.tile([128, 8 * E], f16)        # [p, j*1024 + c]
            boeff_sb = cp.tile([128, 2 * E], f32)     # [p, h*1024 + c], bcast rows

            # persistent double-buffered v tiles: per s-chunk c the 256-col
            # group [v_h0(64) | ones(64) | v_h1(64) | ones(64)]; the 64-wide
            # ones blocks make PV emit the rowsum replicated on partitions
            # 64-127 (reciprocal then yields the broadcast directly).
            v2t = [cp.tile([128, NC * 256], f16, name=f"v2_{i}") for i in range(2)]
            for t in v2t:
                tv = t.rearrange("p (c h z) -> p c h z", c=NC, h=2)
                nc.gpsimd.memset(tv[:, :, :, 64:128], 1.0)

            # ================= software-pipelined batch emission ==========
            # order per batch b:  bands(b) | proj(b+1) fillers | o_proj(b)
            # Emitting proj(b+1) BEFORE o_proj(b) keeps the "acc" PSUM slots
            # available to ready filler chains during attention(b) instead of
            # being grabbed by o_proj tiles that are still blocked on the
            # band-3 normalization (slot-allocation priority inversion).
            def emit_xt_dma(b, fine=False, wqk_rest=None):
                # 8 per-ec tiles so projection chains start as each DMA lands.
                # fine=True (first batch): split per (n-block, ec) so the 2
                # "acc"-bank chains complete incrementally from ~3us.
                xts = [
                    sb.tile([128, S], f16, tag=f"xt{ec}", name=f"xt{ec}", bufs=3)
                    for ec in range(8)
                ]
                xt_dram = xT.ap()[b].rearrange("(ec p) s -> p ec s", p=128)
                if fine:
                    # halves: 728ns transfer > 625ns HWDGE overhead, so the
                    # piece stream stays transfer-bound (32-way splits were
                    # HWDGE-bound and stretched xt(0) to ~21us)
                    for n in range(2):
                        for ec in range(8):
                            nc.sync.dma_start(
                                xts[ec][:, n * 1024 : (n + 1) * 1024],
                                xt_dram[:, ec, n * 1024 : (n + 1) * 1024],
                            )
                            if wqk_rest is not None and ec == 0 and n == 0:
                                wsb, wdr = wqk_rest
                                nc.sync.dma_start(
                                    wsb.rearrange("p (ec c) -> p ec c", ec=8)[
                                        :, 1:8
                                    ],
                                    wdr[:, 1:8],
                                )
                else:
                    for ec in range(8):
                        nc.sync.dma_start(xts[ec], xt_dram[:, ec])
                return xts

            def emit_qkv(b, xts):
                # q/k projection: qkT[d2, s], head-major
                # m=0 -> [q_h0|q_h1] on partitions, m=1 -> [k_h0|k_h1]
                # prologue (b=0): borrow the still-idle att/sc PSUM tags so 6
                # chains progress concurrently as the fine xt pieces land
                tags = ["att", "sc", "acc"] if b == 0 else ["acc"]
                qkT_sb = sb.tile([128, 2 * S], f16, tag="qkT", name="qkT")
                for n in range(S // 512):
                    for m in range(2):
                        pq = ps.tile(
                            [128, 512], f32,
                            tag=tags[(n * 2 + m) % len(tags)], name="pq",
                        )
                        for ec in range(8):
                            nc.tensor.matmul(
                                pq,
                                wqk_sb[:, ec * 256 + m * 128 : ec * 256 + (m + 1) * 128],
                                xts[ec][:, n * 512 : (n + 1) * 512],
                                start=(ec == 0),
                                stop=(ec == 7),
                            )
                        nc.vector.tensor_scalar_add(
                            qkT_sb[:, m * S + n * 512 : m * S + (n + 1) * 512],
                            pq,
                            bqk_sb[:, m : m + 1],
                        )
                return qkT_sb

            def emit_v_group(b, xts, sc4, tag="acc"):
                # v in natural [s, d] layout, 4 s-chunks per PSUM bank
                v2_sb = v2t[b % 2]
                vq = ps.tile([128, 512], f32, tag=tag, name="vq")
                for sub in range(4):
                    c = sc4 * 4 + sub
                    for ec in range(8):
                        nc.tensor.matmul(
                            vq[:, sub * 128 : (sub + 1) * 128],
                            xts[ec][:, c * 128 : (c + 1) * 128],
                            wv_sb[:, ec * 128 : (ec + 1) * 128],
                            start=(ec == 0),
                            stop=(ec == 7),
                        )
                # copy into v2 chunks (skips the ones columns)
                nc.vector.tensor_copy(
                    v2_sb.rearrange("p (c h z) -> p c h z", c=NC, h=2)[
                        :, sc4 * 4 : sc4 * 4 + 4, :, 0:64
                    ],
                    vq.rearrange("p (c h dd) -> p c h dd", c=4, h=2),
                )

            # ---- attention over 4 bands of 512 queries ----
            # HW rule (probe-verified): matmuls from different PE row
            # groups must not write the same PSUM bank -> the two heads'
            # scores go to the two separate banks of one [128,1024] tile,
            # and each head's att accumulator gets its own bank.
            def emit_band(b, qkT_sb, pair, g):
                v2_sb = v2t[b % 2]
                atts = [
                    ps.tile([128, 512], f32, tag="att", name=f"att{h}", bufs=2)
                    for h in range(2)
                ]
                nkj = 4 * g + 4
                for kj in range(nkj):
                    qo = 128 * max(0, kj - 4 * g)
                    scp = ps.tile([128, 1024], f32, tag="sc", name="scp", bufs=2)
                    ex = sb3.tile([128, 1024], f16, tag="ex", name="ex")
                    for h in range(2):
                        nc.tensor.matmul(
                            scp[:, h * 512 + qo : (h + 1) * 512],
                            qkT_sb[h * 64 : (h + 1) * 64,
                                   S + kj * 128 : S + (kj + 1) * 128],
                            qkT_sb[h * 64 : (h + 1) * 64,
                                   g * 512 + qo : (g + 1) * 512],
                            start=True,
                            stop=True,
                            tile_position=(h * 64, 0),
                        )
                    nc.scalar.activation(
                        ex.rearrange("p (h q) -> p h q", h=2)[:, :, qo:512],
                        scp.rearrange("p (h q) -> p h q", h=2)[:, :, qo:512],
                        Exp,
                        scale=SCALE,
                    )
                    if kj >= 4 * g:  # diagonal chunk: zero q < k
                        nc.vector.tensor_mul(
                            ex.rearrange("p (h q) -> p h q", h=2)[
                                :, :, qo : qo + 128
                            ],
                            ex.rearrange("p (h q) -> p h q", h=2)[
                                :, :, qo : qo + 128
                            ],
                            trimask_sb.rearrange("p (h q) -> p h q", h=2),
                        )
                    for h in range(2):
                        nc.tensor.matmul(
                            atts[h][:, qo:512],
                            v2_sb[:, kj * 256 + h * 128 : kj * 256 + (h + 1) * 128],
                            ex[:, h * 512 + qo : (h + 1) * 512],
                            start=(kj == 0),
                            stop=(kj == nkj - 1),
                        )
                # normalize + scatter into o_proj pair layout
                for h in range(2):
                    rb = sb.tile([64, 512], f32, tag="rb", name="rb")
                    nc.vector.reciprocal(rb, atts[h][64:128, :])
                    attv = atts[h].rearrange(
                        "p (u2 w2 pr) -> p u2 w2 pr", u2=32, w2=8
                    )
                    rbv = rb.rearrange(
                        "p (u2 w2 pr) -> p u2 w2 pr", u2=32, w2=8
                    )
                    pav = pair[h].rearrange("p (j u) -> p u j", j=8)
                    for par in range(2):
                        nc.vector.tensor_mul(
                            pav[par * 64 : (par + 1) * 64,
                                g * 32 : (g + 1) * 32, :],
                            attv[0:64, :, :, par : par + 1],
                            rbv[0:64, :, :, par : par + 1],
                        )

            def emit_oproj(b, pair):
                # o_proj: po[u, c] = sum_j pair[h][:, j*128:+128]^T wo_j
                # bias added on DVE during the PSUM->SBUF copy
                for h in range(2):
                    out_sb = sb.tile([128, E], f32, tag="osb", name="osb")
                    for n2 in range(2):
                        po = ps.tile([128, 512], f32, tag="acc", name="po")
                        for j in range(8):
                            nc.tensor.matmul(
                                po,
                                pair[h][:, j * 128 : (j + 1) * 128],
                                wo_sb[:, j * E + n2 * 512 : j * E + (n2 + 1) * 512],
                                start=(j == 0),
                                stop=(j == 7),
                            )
                        nc.vector.tensor_add(
                            out_sb[:, n2 * 512 : (n2 + 1) * 512],
                            po,
                            boeff_sb[:, h * E + n2 * 512 : h * E + (n2 + 1) * 512],
                        )
                        # per-half store shortens the kernel-exit tail;
                        # final batch rides the by-then-idle SP ring
                        ring = nc.sync if b == B - 1 else nc.scalar
                        ring.dma_start(
                            out.ap()[b, h, :, n2 * 512 : (n2 + 1) * 512],
                            out_sb[:, n2 * 512 : (n2 + 1) * 512],
                        )

            def emit_proj(b, xts, interleave=False):
                if interleave:  # (measured slower; kept for reference)
                    # prologue: alternate qk-chain pairs and v chains so both
                    # kinds progress as the fine xt(0) pieces land
                    qkT_sb = sb.tile([128, 2 * S], f16, tag="qkT", name="qkT")
                    tags = ["att", "sc", "acc"]
                    ti = 0
                    for n in range(4):
                        for m in range(2):
                            pq = ps.tile([128, 512], f32, tag=tags[ti % 3],
                                         name="pq")
                            ti += 1
                            for ec in range(8):
                                nc.tensor.matmul(
                                    pq,
                                    wqk_sb[:, ec * 256 + m * 128 : ec * 256 + (m + 1) * 128],
                                    xts[ec][:, n * 512 : (n + 1) * 512],
                                    start=(ec == 0),
                                    stop=(ec == 7),
                                )
                            nc.vector.tensor_scalar_add(
                                qkT_sb[:, m * S + n * 512 : m * S + (n + 1) * 512],
                                pq,
                                bqk_sb[:, m : m + 1],
                            )
                        emit_v_group(b, xts, n, tag=tags[ti % 3])
                        ti += 1
                    return qkT_sb
                qkT_sb = emit_qkv(b, xts)
                vtags = ["sc", "att", "acc", "acc"] if b == 0 else ["acc"] * 4
                for sc4 in range(4):
                    emit_v_group(b, xts, sc4, tag=vtags[sc4])
                return qkT_sb

            # prologue: SP DMA queue order is the schedule —
            # wqk | xt(0) fine | xt(1) | wo+boeff | xt(2) ...
            # small consts ride the Pool/SWDGE path, off the HWDGE queue
            nc.gpsimd.dma_start(bqk_sb, bqk.ap())
            nc.gpsimd.dma_start(trimask_sb, trimask2.ap())
            xtss = {0: emit_xt_dma(0, fine=True, wqk_rest=(wqk_sb, wqk_dram))}
            nc.sync.dma_start(
                wv_sb.rearrange("p (ec c) -> p ec c", ec=8),
                wv.ap().rearrange("(ec p) c -> p ec c", p=128),
            )
            qkts = {0: emit_proj(0, xtss.pop(0))}
            xtss[1] = emit_xt_dma(1)
            nc.sync.dma_start(
                wo_sb.rearrange("p (j c) -> p j c", j=8),
                wo.ap().rearrange("(j p) c -> p j c", p=128),
            )
            nc.sync.dma_start(boeff_sb, boeff.ap())
            pairs = {}
            for b in range(B):
                # pair-layout attn tiles: partition (w%2)*64+d, col (w//2)*128+u
                pairs[b] = [
                    sb.tile([128, 8 * 128], f16, tag=f"pair{h}", name=f"pair{h}",
                            bufs=4)
                    for h in range(2)
                ]
                for g in range(4):
                    emit_band(b, qkts[b], pairs[b], g)
                del qkts[b]
                if b + 2 < B:
                    xtss[b + 2] = emit_xt_dma(b + 2)
                if b + 1 < B:
                    qkts[b + 1] = emit_proj(b + 1, xtss.pop(b + 1))
            # o_proj emitted LAST: ready long before its priority comes up,
            # so it acts as PE filler-of-last-resort (esp. the final batch's
            # ACT-bound attention stretch, which has no next-batch filler)
            for b in range(B):
                emit_oproj(b, pairs[b])

    nc.compile()
    return nc


def _get_program():
    if "nc" not in _CACHE:
        _CACHE["nc"] = _build_program()
    return _CACHE["nc"]


def _host_inputs(x, Wqkv, bqkv, Wo, bo):
    """Per-core input maps (host-side layout prep: cast/slice/fold)."""
    xT = np.ascontiguousarray(x.transpose(0, 2, 1)).astype(np.float16)

    wo16 = Wo.astype(np.float16)

    # fold v-bias through attention (softmax rows sum to 1) into o_proj bias:
    # boeff_h = bo + bv_h @ sum_w Wo[w*64+d, :]
    wsum = Wo.reshape(16, 64, E).sum(axis=0)      # [64, E] float32

    k_idx = np.arange(128)[:, None]
    q_idx = np.arange(128)[None, :]
    tri = (k_idx <= q_idx).astype(np.float16)
    trimask2 = np.concatenate([tri, tri], axis=1)  # [128, 256]

    in_maps = []
    for c in range(NCORES):
        h0, h1 = HPC * c, HPC * c + 1
        qcols = list(range(h0 * 3 * D, h0 * 3 * D + 64)) + list(
            range(h1 * 3 * D, h1 * 3 * D + 64)
        )
        kcols = [cc + 64 for cc in qcols]
        vcols = [cc + 128 for cc in qcols]
        bqk_arr = np.stack(
            [bqkv[qcols].astype(np.float32), bqkv[kcols].astype(np.float32)], axis=1
        )  # [128, 2]
        boeff = np.zeros((128, 2 * E), np.float32)
        for i, h in enumerate((h0, h1)):
            bv = bqkv[h * 3 * D + 128 : h * 3 * D + 192].astype(np.float32)
            boeff[:, i * E : (i + 1) * E] = (bo.astype(np.float32) + bv @ wsum)[None, :]
        in_maps.append(
            {
                "xT": xT,
                "wqk": np.ascontiguousarray(Wqkv[:, qcols + kcols]).astype(np.float16),
                "wv": np.ascontiguousarray(Wqkv[:, vcols]).astype(np.float16),
                "bqk": np.ascontiguousarray(bqk_arr),
                "wo": wo16,
                "boeff": boeff,
                "trimask2": trimask2,
            }
        )
    return in_maps


def kernel(x, mask, Wqkv, bqkv, Wo, bo, _n_cores=NCORES, _trace=False):
    """Full-input, full-output MHA. `mask` is the causal tril mask (hardcoded)."""
    from concourse.bass_utils import run_bass_kernel_spmd

    nc = _get_program()
    in_maps = _host_inputs(
        np.asarray(x), np.asarray(Wqkv), np.asarray(bqkv), np.asarray(Wo), np.asarray(bo)
    )[:_n_cores]
    res = run_bass_kernel_spmd(
        nc, in_maps, core_ids=list(range(_n_cores)), trace=_trace
    )
    out_full = np.zeros((B, S, E), np.float32)
    for c in range(_n_cores):
        o = res.results[c]["out"]  # [B, HPC, 128, E]
        for h in range(HPC):
            g = HPC * c + h
            out_full[:, g * 128 : (g + 1) * 128, :] = o[:, h]
    _CACHE["last_results"] = res
    return out_full

